# revision 9
# baseline (speedup 1.0000x reference)
"""CUR/Nystrom attention kernel for Trainium2 (8 NeuronCores).

Problem: B=2, H=8, N=4096, D=64, m=256 landmarks, 4 Newton-Schulz iters.
Sharding: 16 (b,h) slices -> 2 per core, fully independent.

Key algebraic facts exploited (verified against the reference):
  * The output is invariant to the ORDER of the selected top-256 indices
    (the whole pipeline is permutation-equivariant), so selection only
    needs the top-256 SET.
  * u = softmax_rows(nr_s @ nc^T) directly (no gather of kernel_1 rows).
  * softmax without max-subtraction is safe (logits ~ N(0,1), |x| < 8).
  * All row-softmax normalizations fold into later stages:
      - kernel_3 @ V  -> append a ones-column to V, divide at the end.
      - kernel_1 @ Z  -> append a ones-column to Z, divide at the end.
  * Newton-Schulz runs dual-tracked (V and V^T) so every matmul has a
    natural (pre-transposed) lhsT operand; PSUM evacuations are fused
    with the (aI - X) affine steps.
"""

import os
import sys

sys.path.insert(0, "/opt/trn_rl_repo")

import numpy as np

import concourse.bass as bass  # noqa: E402
import concourse.mybir as mybir  # noqa: E402
import concourse.tile as tile_mod  # noqa: E402
from concourse.tile import TileContext, ScopedClock  # noqa: E402
from concourse.vector_clock import VectorClock  # noqa: E402
from concourse.bass_utils import run_bass_kernel_spmd  # noqa: E402


# -- walrus workaround: this environment's walrus rejects Drain instructions
# carrying more than one sync wait ("Too many sync wait commands"). Split the
# final global-clock waits across multiple Drain instructions.
def _patched_drain_and_barrier(self, tick_clock, wait_clock):
    nc = self.nc
    vc = tick_clock.global_clock
    n = len(vc)
    entries = [(proc, vc[proc]) for proc in range(n) if vc[proc] > 0]
    emitted = False
    for proc, t in entries:
        vvec = [0] * n
        vvec[proc] = t
        d = nc.sync.drain()
        wait_clock.add_sem_waits(d.ins, ScopedClock({None: VectorClock(vvec)}))
        emitted = True
    if not emitted:
        nc.sync.drain()
    nc.all_engine_barrier()
    assert self.sems is not None
    popped = nc._tile_sem_poison_stack.pop()
    assert popped is self._sem_poison
    nc.clear_and_free_semaphores(list(self.sems.allocated().values()))
    nc.all_engine_barrier()


tile_mod.TileContext._drain_and_barrier = _patched_drain_and_barrier


def _split_multi_waits(nc, max_waits=1):
    """This walrus build rejects instructions carrying more than one sync
    wait. Hoist excess waits onto same-engine NOPs inserted just before the
    offending instruction (equivalent semantics: engines are in-order)."""
    fns = nc.m.functions
    fns = fns() if callable(fns) else fns
    ctr = [0]
    for fn in fns:
        blks = fn.blocks
        blks = blks() if callable(blks) else blks
        for b in blks:
            il = b.instructions
            il = il() if callable(il) else il
            new = []
            dirty = False
            for inst in il:
                si = inst.sync_info
                waits = list(si.on_wait) if si is not None else []
                if len(waits) > max_waits:
                    extra, keep = waits[:-max_waits], waits[-max_waits:]
                    for w in extra:
                        ctr[0] += 1
                        nop = mybir.InstNoOp(
                            name=f"I-wsplit-{ctr[0]}", ins=[], outs=[]
                        )
                        nop.engine = inst.engine
                        nop.sync_info = mybir.SyncInfo(on_wait=[w], on_update=[])
                        new.append(nop)
                    inst.sync_info = mybir.SyncInfo(
                        on_wait=keep, on_update=list(si.on_update)
                    )
                    dirty = True
                new.append(inst)
            if dirty:
                b.instructions = new

F32 = mybir.dt.float32
AX = mybir.AxisListType
OP = mybir.AluOpType
ACTF = mybir.ActivationFunctionType

B, H, N, D = 2, 8, 4096, 64
M = 256  # landmarks
NCHUNK = N // 128  # 32
NS_ITERS = 4
N_CORES = 8
BH_PER_CORE = (B * H) // N_CORES  # 2

_TRACE = bool(int(os.environ.get("KERNEL_TRACE", "0")))

if _TRACE:
    try:
        from trn_agent_boot.trn_boot import _ntff_profile_via_ctypes
        from antenv.axon_hooks import set_axon_ntff_profile_hook

        set_axon_ntff_profile_hook(
            _ntff_profile_via_ctypes("/opt/axon/libaxon_pjrt.so")
        )
    except Exception as _e:  # pragma: no cover
        print(f"kernel.py: ntff hook registration failed: {_e}", file=sys.stderr)


# --------------------------------------------------------------------------
# device kernel build
# --------------------------------------------------------------------------


def _emit_bh(nc, pools, consts, io):
    """Emit one (b,h) slice's pipeline."""
    sb_big, sb_ns, sb_small, ps_pair, ps_acc, ps_small = pools
    I7, I15, I13, IDT, ones128, ones_row = consts
    qt_in, kt_in, v, ncst_in, nrst_in, x_out = io

    # ---- loads -----------------------------------------------------------
    # Q^T / K^T (host-transposed) with duplicated halves for 2-way PE row
    # packing.
    qt2 = sb_big.tile([128, N], F32, tag="qt2")
    kt2 = sb_big.tile([128, N], F32, tag="kt2")
    for base in (0, 64):
        nc.sync.dma_start(out=qt2[base : base + 64, :], in_=qt_in[:, :])
        nc.sync.dma_start(out=kt2[base : base + 64, :], in_=kt_in[:, :])

    # V with a ones column appended per chunk: [128, (32, 65)]
    vaug = sb_big.tile([128, NCHUNK, 65], F32, tag="vaug")
    v_r = v.rearrange("(c p) d -> p c d", p=128)
    nc.sync.dma_start(out=vaug[:, :, 0:64], in_=v_r)
    nc.vector.memset(vaug[:, :, 64:65], 1.0)

    # landmark tiles (pre-scaled by 1/8 on host), duplicated halves
    ncst = sb_small.tile([128, M], F32, tag="ncst")
    nrst = sb_small.tile([128, M], F32, tag="nrst")
    for base in (0, 64):
        nc.sync.dma_start(out=ncst[base : base + 64, :], in_=ncst_in[:, :])
        nc.sync.dma_start(out=nrst[base : base + 64, :], in_=nrst_in[:, :])

    # ---- c^T = (Q @ ncs^T)^T  ->  F = exp(c^T)  [128,(2,4096)] ----------
    fstore = sb_big.tile([128, 2, N], F32, tag="fstore")
    for ns in range(N // 512):
        ps = ps_pair.tile([128, 2, 512], F32, tag="pair")
        sl = slice(512 * ns, 512 * ns + 512)
        for hh in (0, 1):
            b0 = 64 * hh
            nc.tensor.matmul(
                out=ps[:, hh, :],
                lhsT=ncst[b0 : b0 + 64, 128 * hh : 128 * hh + 128],
                rhs=qt2[b0 : b0 + 64, sl],
                start=True,
                stop=True,
            )
        nc.scalar.activation(fstore[:, :, sl], ps[:, :, :], ACTF.Exp)

    # ---- r^T = (nr_s @ K^T)^T  ->  E = exp(r^T)  [128,(32,256)] ---------
    estore = sb_big.tile([128, NCHUNK, M], F32, tag="estore")
    for pp in range(NCHUNK // 2):
        ps = ps_pair.tile([128, 2, 512], F32, tag="pair")
        for hh in (0, 1):
            c = 2 * pp + hh
            b0 = 64 * hh
            nc.tensor.matmul(
                out=ps[:, hh, 0:M],
                lhsT=kt2[b0 : b0 + 64, 128 * c : 128 * c + 128],
                rhs=nrst[b0 : b0 + 64, :],
                start=True,
                stop=True,
            )
        nc.scalar.activation(
            estore[:, 2 * pp : 2 * pp + 2, :], ps[:, :, 0:M], ACTF.Exp
        )

    # ---- u = softmax_rows(8 * nrs^T.T @ ncs^T)  [128,(2,256)] -----------
    ps_u = ps_pair.tile([128, 2, 512], F32, tag="pair")
    for hh in (0, 1):
        b0 = 64 * hh
        nc.tensor.matmul(
            out=ps_u[:, hh, 0:M],
            lhsT=nrst[b0 : b0 + 64, 128 * hh : 128 * hh + 128],
            rhs=ncst[b0 : b0 + 64, 0:M],
            start=True,
            stop=True,
        )
    u = sb_ns.tile([128, 2, M], F32, tag="u")
    usum = sb_small.tile([128, 2], F32, tag="usum")
    for hh in (0, 1):
        nc.scalar.activation(
            u[:, hh, :],
            ps_u[:, hh, 0:M],
            ACTF.Exp,
            scale=8.0,
            accum_out=usum[:, hh : hh + 1],
        )
    usum_r = sb_small.tile([128, 2], F32, tag="usum_r")
    nc.vector.reciprocal(usum_r[:, :], usum[:, :])
    for hh in (0, 1):
        nc.vector.tensor_scalar(
            u[:, hh, :], u[:, hh, :], usum_r[:, hh : hh + 1], None, op0=OP.mult
        )

    # ---- u^T via PE transposes ------------------------------------------
    ut = sb_ns.tile([128, 2, M], F32, tag="ut")
    for hi in (0, 1):
        for hj in (0, 1):
            ps_t = ps_small.tile([128, 128], F32, tag="small")
            nc.tensor.transpose(
                out=ps_t[:, :],
                in_=u[:, hi, 128 * hj : 128 * hj + 128],
                identity=IDT[:, :],
            )
            nc.scalar.copy(ut[:, hj, 128 * hi : 128 * hi + 128], ps_t[:, :])

    # ---- NS init: V0 = u^T / max_j colsum(u) ----------------------------
    ps_cs = ps_small.tile([1, M], F32, tag="small")
    for hh in (0, 1):
        nc.tensor.matmul(
            out=ps_cs[:, :],
            lhsT=ones128[:, 0:1],
            rhs=u[:, hh, :],
            start=(hh == 0),
            stop=(hh == 1),
        )
    csmax = sb_small.tile([1, 1], F32, tag="csmax")
    nc.vector.reduce_max(csmax[:, :], ps_cs[:, :], axis=AX.X)
    csinv = sb_small.tile([1, 1], F32, tag="csinv")
    nc.vector.reciprocal(csinv[:, :], csmax[:, :])
    # broadcast scale to [128,1] via K=1 matmul with ones row [1,128]
    ps_bc = ps_small.tile([128, 128], F32, tag="small")
    nc.tensor.matmul(
        out=ps_bc[:, 0:1], lhsT=ones_row[:, :], rhs=csinv[:, :], start=True, stop=True
    )
    scale_col = sb_small.tile([128, 1], F32, tag="scale_col")
    nc.vector.tensor_copy(scale_col[:, :], ps_bc[:, 0:1])

    vns = sb_ns.tile([128, 2, M], F32, tag="vns")
    vnst = sb_ns.tile([128, 2, M], F32, tag="vnst")
    nc.vector.tensor_scalar(
        vns[:, :, :], ut[:, :, :], scale_col[:, :], None, op0=OP.mult
    )
    nc.vector.tensor_scalar(
        vnst[:, :, :], u[:, :, :], scale_col[:, :], None, op0=OP.mult
    )

    # ---- T_aug = E^T-contraction with V_aug (kernel_3 @ V, unnormalized)
    ps_T = []
    for hh in (0, 1):
        pst = ps_acc.tile([128, 65], F32, tag=f"taug{hh}", bufs=1)
        ps_T.append(pst)
        for c in range(NCHUNK):
            nc.tensor.matmul(
                out=pst[:, :],
                lhsT=estore[:, c, 128 * hh : 128 * hh + 128],
                rhs=vaug[:, c, :],
                start=(c == 0),
                stop=(c == NCHUNK - 1),
            )
    # RV = T[:, :64] / T[:, 64]
    rv = sb_small.tile([128, 2, 64], F32, tag="rv")
    rvinv = sb_small.tile([128, 2], F32, tag="rvinv")
    for hh in (0, 1):
        nc.vector.reciprocal(rvinv[:, hh : hh + 1], ps_T[hh][:, 64:65])
        nc.vector.tensor_scalar(
            rv[:, hh, :], ps_T[hh][:, 0:64], rvinv[:, hh : hh + 1], None, op0=OP.mult
        )

    # ---- Newton-Schulz iterations (dual track) --------------------------
    def mm256(out_ps, lhsT_store, rhs_store):
        for mh in (0, 1):
            for kk in (0, 1):
                nc.tensor.matmul(
                    out=out_ps[:, mh, 0:M],
                    lhsT=lhsT_store[:, kk, 128 * mh : 128 * mh + 128],
                    rhs=rhs_store[:, kk, 0:M],
                    start=(kk == 0),
                    stop=(kk == 1),
                )

    vcur, vcurt = vns, vnst
    for it in range(NS_ITERS):
        ps_P = ps_pair.tile([128, 2, 512], F32, tag="pair")
        mm256(ps_P, ut, vcur)
        a_t = sb_ns.tile([128, 2, M], F32, tag="a_t")
        nc.vector.tensor_tensor(
            a_t[:, :, :], I7[:, :, :], ps_P[:, :, 0:M], op=OP.subtract
        )

        ps_PT = ps_pair.tile([128, 2, 512], F32, tag="pair")
        mm256(ps_PT, vcur, ut)
        pt_t = sb_ns.tile([128, 2, M], F32, tag="pt_t")
        nc.scalar.copy(pt_t[:, :, :], ps_PT[:, :, 0:M])

        ps_B = ps_pair.tile([128, 2, 512], F32, tag="pair")
        mm256(ps_B, pt_t, a_t)
        c_t = sb_ns.tile([128, 2, M], F32, tag="c_t")
        nc.vector.tensor_tensor(
            c_t[:, :, :], I15[:, :, :], ps_B[:, :, 0:M], op=OP.subtract
        )

        ps_D = ps_pair.tile([128, 2, 512], F32, tag="pair")
        mm256(ps_D, pt_t, c_t)
        e2_t = sb_ns.tile([128, 2, M], F32, tag="e2_t")
        nc.vector.tensor_tensor(
            e2_t[:, :, :], I13[:, :, :], ps_D[:, :, 0:M], op=OP.subtract
        )

        ps_Vn = ps_pair.tile([128, 2, 512], F32, tag="pair")
        mm256(ps_Vn, vcurt, e2_t)
        vn = sb_ns.tile([128, 2, M], F32, tag="vns")
        nc.scalar.activation(vn[:, :, :], ps_Vn[:, :, 0:M], ACTF.Copy, scale=0.25)

        ps_VnT = ps_pair.tile([128, 2, 512], F32, tag="pair")
        mm256(ps_VnT, e2_t, vcurt)
        vnt = sb_ns.tile([128, 2, M], F32, tag="vnst")
        nc.scalar.activation(vnt[:, :, :], ps_VnT[:, :, 0:M], ACTF.Copy, scale=0.25)

        vcur, vcurt = vn, vnt

    # ---- Z = K2inv @ RV ; Z_aug = [Z | 1]  ------------------------------
    zaug = sb_small.tile([128, 2, 65], F32, tag="zaug")
    for hh in (0, 1):
        ps_z = ps_small.tile([128, 128], F32, tag="small")
        for kk in (0, 1):
            nc.tensor.matmul(
                out=ps_z[:, 0:64],
                lhsT=vcurt[:, kk, 128 * hh : 128 * hh + 128],
                rhs=rv[:, kk, :],
                start=(kk == 0),
                stop=(kk == 1),
            )
        nc.scalar.copy(zaug[:, hh, 0:64], ps_z[:, 0:64])
    nc.vector.memset(zaug[:, :, 64:65], 1.0)

    # ---- X = diag(1/w) (F @ Z_aug) ; 4 chunks per PSUM bank -------------
    x_r = x_out.rearrange("(g cc p) d -> p g cc d", p=128, cc=4)
    for g in range(NCHUNK // 4):
        ps_y = ps_small.tile([128, 4, 65], F32, tag="small")
        for cc in range(4):
            c = 4 * g + cc
            for hh in (0, 1):
                nc.tensor.matmul(
                    out=ps_y[:, cc, :],
                    lhsT=fstore[:, hh, 128 * c : 128 * c + 128],
                    rhs=zaug[:, hh, :],
                    start=(hh == 0),
                    stop=(hh == 1),
                )
        winv = sb_small.tile([128, 4], F32, tag="winv")
        nc.vector.reciprocal(winv[:, :], ps_y[:, :, 64:65])
        xt = sb_small.tile([128, 4, 64], F32, tag="xt", bufs=3)
        nc.vector.tensor_tensor(
            xt[:, :, :],
            ps_y[:, :, 0:64],
            winv.to_broadcast([128, 4, 64]),
            op=OP.mult,
        )
        nc.sync.dma_start(out=x_r[:, g, :, :], in_=xt[:, :, :])


def build_nc():
    nc = bass.Bass()
    ios = []
    for bh in range(BH_PER_CORE):
        io = (
            nc.declare_dram_parameter(f"qt{bh}", [D, N], F32, isOutput=False),
            nc.declare_dram_parameter(f"kt{bh}", [D, N], F32, isOutput=False),
            nc.declare_dram_parameter(f"v{bh}", [N, D], F32, isOutput=False),
            nc.declare_dram_parameter(f"ncst{bh}", [D, M], F32, isOutput=False),
            nc.declare_dram_parameter(f"nrst{bh}", [D, M], F32, isOutput=False),
            nc.declare_dram_parameter(f"x{bh}", [N, D], F32, isOutput=True),
        )
        ios.append(io)

    with TileContext(nc) as tc:
        with (
            tc.tile_pool(name="sb_const", bufs=1) as sb_const,
            tc.tile_pool(name="sb_big", bufs=1) as sb_big,
            tc.tile_pool(name="sb_ns", bufs=2) as sb_ns,
            tc.tile_pool(name="sb_small", bufs=2) as sb_small,
            tc.tile_pool(name="ps_pair", bufs=2, space="PSUM") as ps_pair,
            tc.tile_pool(name="ps_acc", bufs=1, space="PSUM") as ps_acc,
            tc.tile_pool(name="ps_small", bufs=2, space="PSUM") as ps_small,
        ):

            def scaled_identity(val, name):
                t = sb_const.tile([128, 2, M], F32, name=name)
                nc.vector.memset(t[:, :, :], float(val))
                nc.gpsimd.affine_select(
                    out=t[:, :, :],
                    in_=t[:, :, :],
                    pattern=[[-128, 2], [1, M]],
                    compare_op=OP.is_equal,
                    fill=0.0,
                    base=0,
                    channel_multiplier=-1,
                )
                return t

            I7 = scaled_identity(7.0, "I7")
            I15 = scaled_identity(15.0, "I15")
            I13 = scaled_identity(13.0, "I13")
            IDT = sb_const.tile([128, 128], F32, name="IDT")
            nc.vector.memset(IDT[:, :], 1.0)
            nc.gpsimd.affine_select(
                out=IDT[:, :],
                in_=IDT[:, :],
                pattern=[[1, 128]],
                compare_op=OP.is_equal,
                fill=0.0,
                base=0,
                channel_multiplier=-1,
            )
            ones128 = sb_const.tile([128, 1], F32, name="ones128")
            nc.vector.memset(ones128[:, :], 1.0)
            ones_row = sb_const.tile([1, 128], F32, name="ones_row")
            nc.vector.memset(ones_row[:, :], 1.0)

            consts = (I7, I15, I13, IDT, ones128, ones_row)
            pools = (sb_big, sb_ns, sb_small, ps_pair, ps_acc, ps_small)
            for bh in range(BH_PER_CORE):
                _emit_bh(nc, pools, consts, ios[bh])
    _split_multi_waits(nc)
    return nc


# --------------------------------------------------------------------------
# host wrapper
# --------------------------------------------------------------------------

_NC_CACHE = {}


def _get_nc():
    if "nc" not in _NC_CACHE:
        _NC_CACHE["nc"] = build_nc()
    return _NC_CACHE["nc"]


def _host_select(t):  # t: [N, D] -> top-256 row indices by row-sum (any order)
    s = t.sum(axis=-1, dtype=np.float32)
    return np.argpartition(-s, M - 1)[:M]


def kernel(Q, K, V, mask):
    Q = np.asarray(Q, dtype=np.float32)
    K = np.asarray(K, dtype=np.float32)
    V = np.asarray(V, dtype=np.float32)

    nc = _get_nc()

    in_maps = []
    for core in range(N_CORES):
        m = {}
        for j in range(BH_PER_CORE):
            bh = core * BH_PER_CORE + j
            b, h = bh // H, bh % H
            q, k, v = Q[b, h], K[b, h], V[b, h]
            idx_k = _host_select(k)
            idx_q = _host_select(q)
            m[f"qt{j}"] = np.ascontiguousarray(q.T)
            m[f"kt{j}"] = np.ascontiguousarray(k.T)
            m[f"v{j}"] = np.ascontiguousarray(v)
            m[f"ncst{j}"] = np.ascontiguousarray((k[idx_k] / 8.0).T)
            m[f"nrst{j}"] = np.ascontiguousarray((q[idx_q] / 8.0).T)
        in_maps.append(m)

    res = run_bass_kernel_spmd(nc, in_maps, list(range(N_CORES)), trace=_TRACE)
    kernel.last_results = res

    out = np.empty((B, H, N, D), dtype=np.float32)
    for core in range(N_CORES):
        for j in range(BH_PER_CORE):
            bh = core * BH_PER_CORE + j
            b, h = bh // H, bh % H
            out[b, h] = res.results[core][f"x{j}"]
    return out
